# revision 1
# baseline (speedup 1.0000x reference)
"""Causal MHA with RoPE on 8 Trainium2 NeuronCores.

Sharding: core c -> batch b=c//2, head-group g=c%2 (8 heads of 16).
Each core: Q/K/V projections for its 512 head-dims over the full sequence,
causal attention for its 8 heads, partial output projection (its 512 rows
of wo). Host sums the two partial outputs per batch. No collectives.

All matmuls run in float32r (full-rate PE mode, ~1.5e-4 rel err at K=1024).
RoPE is applied via host-permuted wq/wk columns ([evens|odds] per head),
32-row block-swap DMAs and precomputed cos/sin tables.
Softmax skips max-subtraction (scores are O(1) after the 1/8 scale), uses
an additive -1e9 causal mask on diagonal tiles, and gets denominators from
a ones-column appended to V (M=65 AV matmul).
"""

import math

import numpy as np

import concourse.bass as bass
import concourse.mybir as mybir
import concourse.tile as tile
from concourse import bacc
from concourse.bass_utils import run_bass_kernel_spmd
from concourse.masks import make_identity

F32 = mybir.dt.float32
F32R = mybir.dt.float32r

B, S, D, H = 4, 2048, 1024, 16
HD = D // H          # 64
THETA = 10000.0
DH = D // 2          # 512 per-core head dims (8 heads)
NP = 4               # head pairs per core
NTH = 4              # token passes for x^T materialization / projections
THT = S // NTH       # 512 tokens per pass
NQB = 4              # query blocks of 512
QB = S // NQB
NKT = S // 128       # 16 key tiles of 128
SCALE = 1.0 / math.sqrt(HD)
NEG = -1.0e9

_cached = None


def _build():
    nc = bacc.Bacc(None, target_bir_lowering=False)

    x = nc.dram_tensor("x", [S, D], F32, kind="ExternalInput")
    wk = nc.dram_tensor("wk", [D, DH], F32, kind="ExternalInput")
    wq = nc.dram_tensor("wq", [D, DH], F32, kind="ExternalInput")
    wv = nc.dram_tensor("wv", [D, DH], F32, kind="ExternalInput")
    wo = nc.dram_tensor("wo", [DH, D], F32, kind="ExternalInput")
    cosb = nc.dram_tensor("cosb", [128, S], F32, kind="ExternalInput")
    sinb = nc.dram_tensor("sinb", [128, S], F32, kind="ExternalInput")
    outp = nc.dram_tensor("outp", [S, D], F32, kind="ExternalOutput")
    ctxd = nc.dram_tensor("ctxd", [DH, S], F32)  # internal scratch in HBM

    with tile.TileContext(nc) as tc:
        with (
            tc.tile_pool(name="const", bufs=1) as cpool,
            tc.tile_pool(name="kq", bufs=1) as kqpool,
            tc.tile_pool(name="vaug", bufs=1) as vpool,
            tc.tile_pool(name="xt", bufs=8) as xtpool,
            tc.tile_pool(name="stream", bufs=2) as spool,
            tc.tile_pool(name="w512", bufs=8) as wpool,
        ):
            ident = cpool.tile([128, 128], F32, name="ident")
            make_identity(nc, ident)
            tri = cpool.tile([128, 128], F32, name="tri")
            nc.gpsimd.memset(tri, 0.0)
            # tri[i, t] = 0 if t >= i else NEG  (mask k>q inside diagonal tiles)
            nc.gpsimd.affine_select(
                out=tri, in_=tri, compare_op=mybir.AluOpType.is_ge,
                fill=NEG, base=0, pattern=[[1, 128]], channel_multiplier=-1,
            )
            cos_t = cpool.tile([128, S], F32, name="cos_t")
            sin_t = cpool.tile([128, S], F32, name="sin_t")
            nc.sync.dma_start(out=cos_t, in_=cosb[:, :])
            nc.sync.dma_start(out=sin_t, in_=sinb[:, :])

            # K^T / Q^T pair tiles: [128 dims (head 2p | head 2p+1), S tokens]
            kt_tiles = [kqpool.tile([128, S], F32R, name=f"ktp{p}", tag=f"ktp{p}") for p in range(NP)]
            qt_tiles = [kqpool.tile([128, S], F32R, name=f"qtp{p}", tag=f"qtp{p}") for p in range(NP)]
            # V tiles with ones column: [128 tokens, 8 heads, 64+1]
            v_tiles = [vpool.tile([128, 8, HD + 1], F32R, name=f"vt{t}", tag=f"vt{t}") for t in range(NKT)]
            for t in range(NKT):
                # ones column via exp(0*x) = 1
                nc.scalar.activation(
                    v_tiles[t][:, :, HD], cos_t[:, 0:8],
                    mybir.ActivationFunctionType.Exp, scale=0.0,
                )

            with (
                tc.tile_pool(name="pst", bufs=2, space="PSUM") as pst,
                tc.tile_pool(name="psp", bufs=3, space="PSUM") as psp,
            ):
                for th in range(NTH):
                    t0 = th * THT
                    # ---- x^T materialization for this token pass ----
                    xts = []
                    for dc in range(8):
                        xts.append(xtpool.tile([128, THT], F32R, name=f"xt{th}_{dc}", tag="xt"))
                    for tl in range(THT // 128):
                        xl = spool.tile([128, D], F32, name="xl", tag="xl")
                        nc.sync.dma_start(out=xl, in_=x[t0 + tl * 128 : t0 + (tl + 1) * 128, :])
                        for dc in range(8):
                            tp = pst.tile([128, 128], F32, name="tp", tag="tp")
                            nc.tensor.transpose(tp, xl[:, dc * 128 : (dc + 1) * 128], ident)
                            nc.scalar.copy(xts[dc][:, tl * 128 : (tl + 1) * 128], tp)

                    # ---- K^T / Q^T projections + RoPE for this token pass ----
                    for wmat, dst in ((wk, kt_tiles), (wq, qt_tiles)):
                        wr = []
                        for dc in range(8):
                            wf = spool.tile([128, DH], F32, name="wf", tag="wf")
                            nc.sync.dma_start(out=wf, in_=wmat[dc * 128 : (dc + 1) * 128, :])
                            wrc = wpool.tile([128, DH], F32R, name="wrc", tag="w512")
                            nc.scalar.copy(wrc, wf)
                            wr.append(wrc)
                        for p in range(NP):
                            acc = psp.tile([128, THT], F32, name="acc", tag="pp")
                            for dc in range(8):
                                nc.tensor.matmul(
                                    acc, wr[dc][:, p * 128 : (p + 1) * 128], xts[dc],
                                    start=(dc == 0), stop=(dc == 7),
                                )
                            # rope: out = raw*cos + swap(raw)*sin
                            raw = spool.tile([128, THT], F32, name="raw", tag="raw")
                            nc.scalar.copy(raw, acc)
                            swp = spool.tile([128, THT], F32, name="swp", tag="swp")
                            nc.sync.dma_start(out=swp[0:32, :], in_=raw[32:64, :])
                            nc.sync.dma_start(out=swp[32:64, :], in_=raw[0:32, :])
                            nc.sync.dma_start(out=swp[64:96, :], in_=raw[96:128, :])
                            nc.sync.dma_start(out=swp[96:128, :], in_=raw[64:96, :])
                            nc.vector.tensor_mul(raw, raw, cos_t[:, t0 : t0 + THT])
                            nc.vector.tensor_mul(swp, swp, sin_t[:, t0 : t0 + THT])
                            nc.vector.tensor_add(dst[p][:, t0 : t0 + THT], raw, swp)

                    # ---- V projection for this token pass ----
                    wvr = []
                    for dc in range(8):
                        wf = spool.tile([128, DH], F32, name="wvf", tag="wf")
                        nc.sync.dma_start(out=wf, in_=wv[dc * 128 : (dc + 1) * 128, :])
                        wrc = wpool.tile([128, DH], F32R, name="wvr", tag="w512")
                        nc.scalar.copy(wrc, wf)
                        wvr.append(wrc)
                    for tl in range(THT // 128):
                        acc = psp.tile([128, DH], F32, name="vacc", tag="pp")
                        for dc in range(8):
                            nc.tensor.matmul(
                                acc, xts[dc][:, tl * 128 : (tl + 1) * 128], wvr[dc],
                                start=(dc == 0), stop=(dc == 7),
                            )
                        vt = v_tiles[th * (THT // 128) + tl]
                        # strided write: psum [128, 8*64] -> v_aug[:, h, 0:64]
                        nc.scalar.copy(
                            vt[:, :, 0:HD],
                            acc.rearrange("a (h d) -> a h d", h=8),
                        )

            # ---------------- attention ----------------
            with (
                tc.tile_pool(name="pss", bufs=2, space="PSUM") as pss,
                tc.tile_pool(name="psc", bufs=2, space="PSUM") as psc,
            ):
                for p in range(NP):
                    ktp, qtp = kt_tiles[p], qt_tiles[p]
                    for qb in range(NQB):
                        q0 = qb * QB
                        nk = 4 * qb + 4
                        pse = psc.tile([HD + 1, QB], F32, name="pse", tag="ctxe")
                        pso = psc.tile([HD + 1, QB], F32, name="pso", tag="ctxo")
                        for kt in range(nk):
                            dj = kt - (nk - 4)
                            qoff = 128 * dj if dj > 0 else 0
                            n = QB - qoff
                            psa = pss.tile([128, n], F32, name="psa", tag="scA")
                            psb = pss.tile([128, n], F32, name="psb", tag="scB")
                            ksl = slice(kt * 128, (kt + 1) * 128)
                            qsl = slice(q0 + qoff, q0 + QB)
                            nc.tensor.matmul(psa, ktp[0:64, ksl], qtp[0:64, qsl])
                            nc.tensor.matmul(psb, ktp[64:128, ksl], qtp[64:128, qsl])
                            if dj >= 0:
                                nc.vector.tensor_add(psa[:, 0:128], psa[:, 0:128], tri)
                                nc.vector.tensor_add(psb[:, 0:128], psb[:, 0:128], tri)
                            ea = spool.tile([128, QB], F32R, name="ea", tag="ea")
                            eb = spool.tile([128, QB], F32R, name="eb", tag="eb")
                            nc.scalar.activation(
                                ea[:, qoff:QB], psa, mybir.ActivationFunctionType.Exp, scale=SCALE)
                            nc.scalar.activation(
                                eb[:, qoff:QB], psb, mybir.ActivationFunctionType.Exp, scale=SCALE)
                            vt = v_tiles[kt]
                            nc.tensor.matmul(
                                pse[:, qoff:QB], vt[:, 2 * p, :], ea[:, qoff:QB],
                                start=(kt == 0), stop=(kt == nk - 1))
                            nc.tensor.matmul(
                                pso[:, qoff:QB], vt[:, 2 * p + 1, :], eb[:, qoff:QB],
                                start=(kt == 0), stop=(kt == nk - 1))
                        for par, psx in ((0, pse), (1, pso)):
                            hloc = 2 * p + par
                            s0 = spool.tile([1, QB], F32, name="s0", tag="s0")
                            nc.scalar.copy(s0[0:1, :], psx[HD : HD + 1, :])
                            inv0 = spool.tile([1, QB], F32, name="inv0", tag="inv0")
                            nc.vector.reciprocal(inv0[0:1, :], s0[0:1, :])
                            bc = spool.tile([HD, QB], F32, name="bc", tag="bc")
                            nc.gpsimd.partition_broadcast(bc, inv0[0:1, :])
                            cn = spool.tile([HD, QB], F32, name="cn", tag="cn")
                            nc.vector.tensor_mul(cn, psx[0:HD, :], bc)
                            nc.sync.dma_start(
                                out=ctxd[hloc * HD : (hloc + 1) * HD, q0 : q0 + QB], in_=cn)

            # ---------------- output projection ----------------
            with tc.tile_pool(name="pso2", bufs=2, space="PSUM") as pso2p:
                for nn in range(2):
                    wor = []
                    for pc in range(4):
                        wf = spool.tile([128, 512], F32, name="wof", tag="wf")
                        nc.sync.dma_start(
                            out=wf, in_=wo[pc * 128 : (pc + 1) * 128, nn * 512 : (nn + 1) * 512])
                        wrc = wpool.tile([128, 512], F32R, name="wor", tag="w512")
                        nc.scalar.copy(wrc, wf)
                        wor.append(wrc)
                    for t in range(NKT):
                        acc = pso2p.tile([128, 512], F32, name="oacc", tag="oacc")
                        for pc in range(4):
                            cf = spool.tile([128, 128], F32, name="cf", tag="cf")
                            nc.sync.dma_start(
                                out=cf, in_=ctxd[pc * 128 : (pc + 1) * 128, t * 128 : (t + 1) * 128])
                            cr = spool.tile([128, 128], F32R, name="cr", tag="cr", bufs=3)
                            nc.scalar.copy(cr, cf)
                            nc.tensor.matmul(acc, cr, wor[pc], start=(pc == 0), stop=(pc == 3))
                        osb = spool.tile([128, 512], F32, name="osb", tag="osb")
                        nc.scalar.copy(osb, acc)
                        nc.sync.dma_start(
                            out=outp[t * 128 : (t + 1) * 128, nn * 512 : (nn + 1) * 512], in_=osb)

    nc.compile()
    return nc


def _host_tables(token_positions):
    pos = np.asarray(token_positions, dtype=np.float64)
    inv_freq = np.exp(np.arange(0, HD, 2, dtype=np.float64) * (-math.log(THETA) / HD))  # [32]
    ang = pos[:, None] * inv_freq[None, :]  # [S, 32]
    cos = np.cos(ang).astype(np.float32).T  # [32, S]
    sin = np.sin(ang).astype(np.float32).T
    # pair-tile row layout: [head_even: 32 evens | 32 odds][head_odd: same]
    C = np.empty((128, S), np.float32)
    Sx = np.empty((128, S), np.float32)
    for half in range(2):
        r0 = 64 * half
        C[r0 : r0 + 32] = cos
        C[r0 + 32 : r0 + 64] = cos
        Sx[r0 : r0 + 32] = -sin
        Sx[r0 + 32 : r0 + 64] = sin
    return C, Sx


def kernel(in_features, token_positions, wq, wk, wv, wo):
    global _cached
    if _cached is None:
        _cached = _build()
    nc = _cached

    x = np.ascontiguousarray(in_features, dtype=np.float32)
    # permute wq/wk columns within each head: [evens | odds]
    perm = np.concatenate(
        [64 * h + np.concatenate([np.arange(0, 64, 2), np.arange(1, 64, 2)]) for h in range(H)])
    wqp = np.ascontiguousarray(wq[:, perm], dtype=np.float32)
    wkp = np.ascontiguousarray(wk[:, perm], dtype=np.float32)
    wv = np.ascontiguousarray(wv, dtype=np.float32)
    wo = np.ascontiguousarray(wo, dtype=np.float32)
    C, Sx = _host_tables(token_positions)

    in_maps = []
    for c in range(8):
        b, g = c // 2, c % 2
        sl = slice(g * DH, (g + 1) * DH)
        in_maps.append({
            "x": np.ascontiguousarray(x[b]),
            "wq": np.ascontiguousarray(wqp[:, sl]),
            "wk": np.ascontiguousarray(wkp[:, sl]),
            "wv": np.ascontiguousarray(wv[:, sl]),
            "wo": np.ascontiguousarray(wo[sl, :]),
            "cosb": C,
            "sinb": Sx,
        })
    res = run_bass_kernel_spmd(nc, in_maps, core_ids=list(range(8)))
    out = np.empty((B, S, D), np.float32)
    for b in range(B):
        out[b] = res.results[2 * b]["outp"] + res.results[2 * b + 1]["outp"]
    return out


# revision 2
# speedup vs baseline: 2.1611x; 2.1611x over previous
"""Causal MHA with RoPE on 8 Trainium2 NeuronCores.

Sharding: core c -> batch b=c//2, head-group g=c%2 (8 heads of 16).
Each core: Q/K/V projections for its 512 head-dims over the full sequence,
causal attention for its 8 heads, partial output projection (its 512 rows
of wo). Host sums the two partial outputs per batch. No collectives.

All matmuls run in float32r (full-rate PE mode, ~1.5e-4 rel err at K=1024).
RoPE is applied via host-permuted wq/wk columns ([evens|odds] per head),
32-row block-swap DMAs and precomputed cos/sin tables.
Softmax skips max-subtraction (scores are O(1) after the 1/8 scale), uses
an additive -1e9 causal mask on diagonal tiles, and gets denominators from
a ones-column appended to V (M=65 AV matmul).
"""

import math

import numpy as np

import concourse.bass as bass
import concourse.mybir as mybir
import concourse.tile as tile
from concourse import bacc
from concourse.bass_utils import run_bass_kernel_spmd
from concourse.masks import make_identity

F32 = mybir.dt.float32
F32R = mybir.dt.float32r

B, S, D, H = 4, 2048, 1024, 16
HD = D // H          # 64
THETA = 10000.0
DH = D // 2          # 512 per-core head dims (8 heads)
NP = 4               # head pairs per core
NTH = 4              # token passes for x^T materialization / projections
THT = S // NTH       # 512 tokens per pass
NQB = 4              # query blocks of 512
QB = S // NQB
NKT = S // 128       # 16 key tiles of 128
SCALE = 1.0 / math.sqrt(HD)
NEG = -1.0e9

_cached = None


def _build():
    nc = bacc.Bacc(None, target_bir_lowering=False)

    x = nc.dram_tensor("x", [S, D], F32, kind="ExternalInput")
    wk = nc.dram_tensor("wk", [D, DH], F32, kind="ExternalInput")
    wq = nc.dram_tensor("wq", [D, DH], F32, kind="ExternalInput")
    wv = nc.dram_tensor("wv", [D, DH], F32, kind="ExternalInput")
    wo = nc.dram_tensor("wo", [DH, D], F32, kind="ExternalInput")
    cosb = nc.dram_tensor("cosb", [128, S], F32, kind="ExternalInput")
    sinb = nc.dram_tensor("sinb", [128, S], F32, kind="ExternalInput")
    outp = nc.dram_tensor("outp", [S, D], F32, kind="ExternalOutput")
    ctxd = nc.dram_tensor("ctxd", [DH, S], F32)  # internal scratch in HBM

    with tile.TileContext(nc) as tc:
        with (
            tc.tile_pool(name="const", bufs=1) as cpool,
            tc.tile_pool(name="kq", bufs=1) as kqpool,
            tc.tile_pool(name="vaug", bufs=1) as vpool,
            tc.tile_pool(name="xt", bufs=8) as xtpool,
            tc.tile_pool(name="stream", bufs=2) as spool,
            tc.tile_pool(name="w512", bufs=8) as wpool,
        ):
            ident = cpool.tile([128, 128], F32, name="ident")
            make_identity(nc, ident)
            tri = cpool.tile([128, 128], F32, name="tri")
            nc.gpsimd.memset(tri, 0.0)
            # tri[i, t] = 0 if t >= i else NEG  (mask k>q inside diagonal tiles)
            nc.gpsimd.affine_select(
                out=tri, in_=tri, compare_op=mybir.AluOpType.is_ge,
                fill=NEG, base=0, pattern=[[1, 128]], channel_multiplier=-1,
            )
            cos_t = cpool.tile([128, S], F32, name="cos_t")
            sin_t = cpool.tile([128, S], F32, name="sin_t")
            nc.sync.dma_start(out=cos_t, in_=cosb[:, :])
            nc.sync.dma_start(out=sin_t, in_=sinb[:, :])

            # K^T / Q^T pair tiles: [128 dims (head 2p | head 2p+1), S tokens]
            kt_tiles = [kqpool.tile([128, S], F32R, name=f"ktp{p}", tag=f"ktp{p}") for p in range(NP)]
            qt_tiles = [kqpool.tile([128, S], F32R, name=f"qtp{p}", tag=f"qtp{p}") for p in range(NP)]
            # V tiles with ones column: [128 tokens, 8 heads, 64+1]
            v_tiles = [vpool.tile([128, 8, HD + 1], F32R, name=f"vt{t}", tag=f"vt{t}") for t in range(NKT)]
            for t in range(NKT):
                # ones column via exp(0*x) = 1
                nc.scalar.activation(
                    v_tiles[t][:, :, HD], cos_t[:, 0:8],
                    mybir.ActivationFunctionType.Exp, scale=0.0,
                )

            with (
                tc.tile_pool(name="pst", bufs=2, space="PSUM") as pst,
                tc.tile_pool(name="psp", bufs=3, space="PSUM") as psp,
            ):
                for th in range(NTH):
                    t0 = th * THT
                    # ---- x^T materialization for this token pass ----
                    xts = []
                    for dc in range(8):
                        xts.append(xtpool.tile([128, THT], F32R, name=f"xt{th}_{dc}", tag="xt"))
                    for tl in range(THT // 128):
                        xl = spool.tile([128, D], F32, name="xl", tag="xl")
                        nc.sync.dma_start(out=xl, in_=x[t0 + tl * 128 : t0 + (tl + 1) * 128, :])
                        for dc in range(8):
                            tp = pst.tile([128, 128], F32, name="tp", tag="tp")
                            nc.tensor.transpose(tp, xl[:, dc * 128 : (dc + 1) * 128], ident)
                            nc.vector.tensor_copy(xts[dc][:, tl * 128 : (tl + 1) * 128], tp)

                    # ---- K^T / Q^T projections + RoPE for this token pass ----
                    for wmat, dst in ((wk, kt_tiles), (wq, qt_tiles)):
                        wr = []
                        for dc in range(8):
                            wf = spool.tile([128, DH], F32, name="wf", tag="wf")
                            nc.sync.dma_start(out=wf, in_=wmat[dc * 128 : (dc + 1) * 128, :])
                            wrc = wpool.tile([128, DH], F32R, name="wrc", tag="w512")
                            nc.gpsimd.tensor_copy(wrc, wf)
                            wr.append(wrc)
                        for p in range(NP):
                            acc = psp.tile([128, THT], F32, name="acc", tag="pp")
                            for dc in range(8):
                                nc.tensor.matmul(
                                    acc, wr[dc][:, p * 128 : (p + 1) * 128], xts[dc],
                                    start=(dc == 0), stop=(dc == 7),
                                )
                            # rope: out = raw*cos + swap(raw)*sin
                            raw = spool.tile([128, THT], F32, name="raw", tag="raw")
                            nc.vector.tensor_copy(raw, acc)
                            swp = spool.tile([128, THT], F32, name="swp", tag="swp")
                            nc.sync.dma_start(out=swp[0:32, :], in_=raw[32:64, :])
                            nc.sync.dma_start(out=swp[32:64, :], in_=raw[0:32, :])
                            nc.sync.dma_start(out=swp[64:96, :], in_=raw[96:128, :])
                            nc.sync.dma_start(out=swp[96:128, :], in_=raw[64:96, :])
                            nc.vector.tensor_mul(raw, raw, cos_t[:, t0 : t0 + THT])
                            nc.vector.tensor_mul(swp, swp, sin_t[:, t0 : t0 + THT])
                            nc.vector.tensor_add(dst[p][:, t0 : t0 + THT], raw, swp)

                    # ---- V projection for this token pass ----
                    wvr = []
                    for dc in range(8):
                        wf = spool.tile([128, DH], F32, name="wvf", tag="wf")
                        nc.sync.dma_start(out=wf, in_=wv[dc * 128 : (dc + 1) * 128, :])
                        wrc = wpool.tile([128, DH], F32R, name="wvr", tag="w512")
                        nc.gpsimd.tensor_copy(wrc, wf)
                        wvr.append(wrc)
                    for tl in range(THT // 128):
                        acc = psp.tile([128, DH], F32, name="vacc", tag="pp")
                        for dc in range(8):
                            nc.tensor.matmul(
                                acc, xts[dc][:, tl * 128 : (tl + 1) * 128], wvr[dc],
                                start=(dc == 0), stop=(dc == 7),
                            )
                        vt = v_tiles[th * (THT // 128) + tl]
                        # strided write: psum [128, 8*64] -> v_aug[:, h, 0:64]
                        nc.vector.tensor_copy(
                            vt[:, :, 0:HD],
                            acc.rearrange("a (h d) -> a h d", h=8),
                        )

            # ---------------- attention ----------------
            with (
                tc.tile_pool(name="pss", bufs=2, space="PSUM") as pss,
                tc.tile_pool(name="psc", bufs=2, space="PSUM") as psc,
            ):
                for p in range(NP):
                    ktp, qtp = kt_tiles[p], qt_tiles[p]
                    for qb in range(NQB):
                        q0 = qb * QB
                        nk = 4 * qb + 4
                        pse = psc.tile([HD + 1, QB], F32, name="pse", tag="ctxe")
                        pso = psc.tile([HD + 1, QB], F32, name="pso", tag="ctxo")
                        for kt in range(nk):
                            dj = kt - (nk - 4)
                            qoff = 128 * dj if dj > 0 else 0
                            n = QB - qoff
                            psa = pss.tile([128, n], F32, name="psa", tag="scA")
                            psb = pss.tile([128, n], F32, name="psb", tag="scB")
                            ksl = slice(kt * 128, (kt + 1) * 128)
                            qsl = slice(q0 + qoff, q0 + QB)
                            nc.tensor.matmul(psa, ktp[0:64, ksl], qtp[0:64, qsl])
                            nc.tensor.matmul(psb, ktp[64:128, ksl], qtp[64:128, qsl])
                            if dj >= 0:
                                nc.vector.tensor_add(psa[:, 0:128], psa[:, 0:128], tri)
                                nc.vector.tensor_add(psb[:, 0:128], psb[:, 0:128], tri)
                            ea = spool.tile([128, QB], F32R, name="ea", tag="ea")
                            eb = spool.tile([128, QB], F32R, name="eb", tag="eb")
                            nc.scalar.activation(
                                ea[:, qoff:QB], psa, mybir.ActivationFunctionType.Exp, scale=SCALE)
                            nc.scalar.activation(
                                eb[:, qoff:QB], psb, mybir.ActivationFunctionType.Exp, scale=SCALE)
                            vt = v_tiles[kt]
                            nc.tensor.matmul(
                                pse[:, qoff:QB], vt[:, 2 * p, :], ea[:, qoff:QB],
                                start=(kt == 0), stop=(kt == nk - 1))
                            nc.tensor.matmul(
                                pso[:, qoff:QB], vt[:, 2 * p + 1, :], eb[:, qoff:QB],
                                start=(kt == 0), stop=(kt == nk - 1))
                        for par, psx in ((0, pse), (1, pso)):
                            hloc = 2 * p + par
                            s0 = spool.tile([1, QB], F32, name="s0", tag="s0")
                            nc.vector.tensor_copy(s0[0:1, :], psx[HD : HD + 1, :])
                            inv0 = spool.tile([1, QB], F32, name="inv0", tag="inv0")
                            nc.vector.reciprocal(inv0[0:1, :], s0[0:1, :])
                            bc = spool.tile([HD, QB], F32, name="bc", tag="bc")
                            nc.gpsimd.partition_broadcast(bc, inv0[0:1, :])
                            cn = spool.tile([HD, QB], F32, name="cn", tag="cn")
                            nc.vector.tensor_mul(cn, psx[0:HD, :], bc)
                            nc.sync.dma_start(
                                out=ctxd[hloc * HD : (hloc + 1) * HD, q0 : q0 + QB], in_=cn)

            # ---------------- output projection ----------------
            with tc.tile_pool(name="pso2", bufs=2, space="PSUM") as pso2p:
                for nn in range(2):
                    wor = []
                    for pc in range(4):
                        wf = spool.tile([128, 512], F32, name="wof", tag="wf")
                        nc.sync.dma_start(
                            out=wf, in_=wo[pc * 128 : (pc + 1) * 128, nn * 512 : (nn + 1) * 512])
                        wrc = wpool.tile([128, 512], F32R, name="wor", tag="w512")
                        nc.gpsimd.tensor_copy(wrc, wf)
                        wor.append(wrc)
                    for t in range(NKT):
                        acc = pso2p.tile([128, 512], F32, name="oacc", tag="oacc")
                        for pc in range(4):
                            cf = spool.tile([128, 128], F32, name="cf", tag="cf")
                            nc.sync.dma_start(
                                out=cf, in_=ctxd[pc * 128 : (pc + 1) * 128, t * 128 : (t + 1) * 128])
                            cr = spool.tile([128, 128], F32R, name="cr", tag="cr", bufs=3)
                            nc.gpsimd.tensor_copy(cr, cf)
                            nc.tensor.matmul(acc, cr, wor[pc], start=(pc == 0), stop=(pc == 3))
                        osb = spool.tile([128, 512], F32, name="osb", tag="osb")
                        nc.vector.tensor_copy(osb, acc)
                        nc.sync.dma_start(
                            out=outp[t * 128 : (t + 1) * 128, nn * 512 : (nn + 1) * 512], in_=osb)

    nc.compile()
    return nc


def _host_tables(token_positions):
    pos = np.asarray(token_positions, dtype=np.float64)
    inv_freq = np.exp(np.arange(0, HD, 2, dtype=np.float64) * (-math.log(THETA) / HD))  # [32]
    ang = pos[:, None] * inv_freq[None, :]  # [S, 32]
    cos = np.cos(ang).astype(np.float32).T  # [32, S]
    sin = np.sin(ang).astype(np.float32).T
    # pair-tile row layout: [head_even: 32 evens | 32 odds][head_odd: same]
    C = np.empty((128, S), np.float32)
    Sx = np.empty((128, S), np.float32)
    for half in range(2):
        r0 = 64 * half
        C[r0 : r0 + 32] = cos
        C[r0 + 32 : r0 + 64] = cos
        Sx[r0 : r0 + 32] = -sin
        Sx[r0 + 32 : r0 + 64] = sin
    return C, Sx


def kernel(in_features, token_positions, wq, wk, wv, wo):
    global _cached
    if _cached is None:
        _cached = _build()
    nc = _cached

    x = np.ascontiguousarray(in_features, dtype=np.float32)
    # permute wq/wk columns within each head: [evens | odds]
    perm = np.concatenate(
        [64 * h + np.concatenate([np.arange(0, 64, 2), np.arange(1, 64, 2)]) for h in range(H)])
    wqp = np.ascontiguousarray(wq[:, perm], dtype=np.float32)
    wkp = np.ascontiguousarray(wk[:, perm], dtype=np.float32)
    wv = np.ascontiguousarray(wv, dtype=np.float32)
    wo = np.ascontiguousarray(wo, dtype=np.float32)
    C, Sx = _host_tables(token_positions)

    in_maps = []
    for c in range(8):
        b, g = c // 2, c % 2
        sl = slice(g * DH, (g + 1) * DH)
        in_maps.append({
            "x": np.ascontiguousarray(x[b]),
            "wq": np.ascontiguousarray(wqp[:, sl]),
            "wk": np.ascontiguousarray(wkp[:, sl]),
            "wv": np.ascontiguousarray(wv[:, sl]),
            "wo": np.ascontiguousarray(wo[sl, :]),
            "cosb": C,
            "sinb": Sx,
        })
    res = run_bass_kernel_spmd(nc, in_maps, core_ids=list(range(8)))
    out = np.empty((B, S, D), np.float32)
    for b in range(B):
        out[b] = res.results[2 * b]["outp"] + res.results[2 * b + 1]["outp"]
    return out
